# revision 25
# baseline (speedup 1.0000x reference)
"""Multi-head attention TRN2 kernel (B=2, S=2048, D=1024, H=16, DK=64).

Sharding: 8 cores; core c handles batch c//4 and heads 4*(c%4) .. 4*(c%4)+3.
Host pre-transposes activations/weights into partition-major layouts (pure
layout marshalling, no FLOPs); each core computes q/k/v projections for its
4 heads, full attention for those heads (including the [4, S, S]
attention-weight output), and a partial output projection. Host sums the 4
partial outputs per batch and adds b_o.
"""

import sys
import types

sys.path.insert(0, "/opt/trn_rl_repo")

import numpy as np

import concourse.bass as bass
import concourse.tile as tile
from concourse import bacc, mybir
from concourse.bass_utils import run_bass_kernel_spmd

B, S, D, H = 2, 2048, 1024, 16
DK = D // H          # 64
HPC = H // 4         # heads per core = 4
DC = HPC * DK        # 256 context dims per core
P = 128
KT = D // P          # 8 contraction tiles for projections

F32 = mybir.dt.float32
F32R = mybir.dt.float32r
BF16 = mybir.dt.bfloat16
FP = mybir.dt.float32

MMD = F32R  # dtype for f32-storage tiles consumed by matmul


def _install_ntff_hook():
    """run_bass_kernel_spmd(trace=True) needs antenv.axon_hooks, absent in
    this image. Recreate it and register the ctypes-based NTFF hook."""
    import antenv

    if "antenv.axon_hooks" in sys.modules:
        return
    mod = types.ModuleType("antenv.axon_hooks")
    mod._hook = None

    def set_axon_ntff_profile_hook(h):
        mod._hook = h

    def get_axon_ntff_profile_hook():
        return mod._hook

    mod.set_axon_ntff_profile_hook = set_axon_ntff_profile_hook
    mod.get_axon_ntff_profile_hook = get_axon_ntff_profile_hook
    sys.modules["antenv.axon_hooks"] = mod
    antenv.axon_hooks = mod
    try:
        from trn_agent_boot.trn_boot import _ntff_profile_via_ctypes

        hook = _ntff_profile_via_ctypes("/opt/axon/libaxon_pjrt.so")
        if hook is not None:
            set_axon_ntff_profile_hook(hook)
    except Exception:
        pass


def build_nc():
    nc = bacc.Bacc("TRN2", target_bir_lowering=False, debug=False)

    # host-marshalled, partition-major: [p, half, kt, s'] with D = kt*128 + p
    # and s = half*1024 + s'
    xqt = nc.dram_tensor("xqt", (P, 2, KT, S // 2), MMD, kind="ExternalInput")
    xkt = nc.dram_tensor("xkt", (P, 2, KT, S // 2), MMD, kind="ExternalInput")
    xvt = nc.dram_tensor("xvt", (P, 2, KT, S // 2), MMD, kind="ExternalInput")
    # weights [p, kt, dout] with D = kt*128 + p
    wqt = nc.dram_tensor("wqt", (P, KT, DC), MMD, kind="ExternalInput")
    wkt = nc.dram_tensor("wkt", (P, KT, DC), MMD, kind="ExternalInput")
    wvt = nc.dram_tensor("wvt", (P, KT, DC), MMD, kind="ExternalInput")
    # W_o slice [p, g, n] with d_core = g*128 + p
    wot = nc.dram_tensor("wot", (P, DC // P, D), MMD, kind="ExternalInput")
    bq = nc.dram_tensor("bq", (P, DC // P), FP, kind="ExternalInput")
    bk = nc.dram_tensor("bk", (P, DC // P), FP, kind="ExternalInput")
    bv = nc.dram_tensor("bv", (P, DC // P), FP, kind="ExternalInput")

    attw = nc.dram_tensor("attw", (HPC, S, S), FP, kind="ExternalOutput")
    outp = nc.dram_tensor("outp", (S, D), FP, kind="ExternalOutput")

    NCH = S // P       # 16 s_q chunks of 128 (pass A)
    NBLK = 4           # s_q blocks of 512 (pass B)
    BLK = S // NBLK    # 512
    NKT = S // P       # 16 s_k tiles
    VW = DK + 1        # 65: V columns + ones column

    with tile.TileContext(nc) as tc:
        with (
            tc.tile_pool(name="persist", bufs=1) as persist,
        ):
            # weights
            w_sb = {}
            for name, w in (("q", wqt), ("k", wkt), ("v", wvt)):
                t = persist.tile([P, KT, DC], MMD, tag=f"w{name}")
                nc.sync.dma_start(t[:], w[:])
                w_sb[name] = t
            wot_sb = persist.tile([P, DC // P, D], MMD, tag="wot")
            nc.sync.dma_start(wot_sb[:], wot[:])

            bq_sb = persist.tile([P, DC // P], FP, tag="bq")
            nc.sync.dma_start(bq_sb[:], bq[:])
            bk_sb = persist.tile([P, DC // P], FP, tag="bk")
            nc.sync.dma_start(bk_sb[:], bk[:])
            bv_sb = persist.tile([P, DC // P], FP, tag="bv")
            nc.sync.dma_start(bv_sb[:], bv[:])

            ones_sb = persist.tile([1, DK], FP, tag="ones")
            nc.vector.memset(ones_sb[:], 1.0)

            # persistent activations
            qT = persist.tile([P, DC // P, S], MMD, tag="qT")   # [d, s]
            kT = persist.tile([P, DC // P, S], MMD, tag="kT")
            # bf16 copies for pass-B score matmuls (f32r moving operands
            # stream at half rate; pass B feeds bf16 PV anyway)
            qTb = persist.tile([P, DC // P, S], BF16, tag="qTb")
            kTb = persist.tile([P, DC // P, S], BF16, tag="kTb")
            # [s_k(128), kt, h, VW] bf16: V columns + ones column per head
            v_aug = persist.tile([P, NKT, HPC, VW], BF16, tag="vaug")
            nc.vector.memset(v_aug[:, :, :, DK], 1.0)
            ctxT = persist.tile([P, DC // P, S], MMD, tag="ctxT")

            # ---------------- Phase 1: projections ----------------------
            with (
                tc.tile_pool(name="p1x", bufs=2) as p1x,
                tc.tile_pool(name="p1ps", bufs=2, space="PSUM") as p1ps,
                tc.tile_pool(name="p1psv", bufs=2, space="PSUM") as p1psv,
            ):
                SH = S // 2
                for name, x in (("q", xqt), ("k", xkt), ("v", xvt)):
                    for half in range(2):
                        xT = p1x.tile([P, KT, SH], MMD, tag="xT")
                        nc.sync.dma_start(xT[:], x[:, half, :, :])
                        if name in ("q", "k"):
                            dst = qT if name == "q" else kT
                            dstb = qTb if name == "q" else kTb
                            bias = bq_sb if name == "q" else bk_sb
                            for sb in range(SH // 512):
                                s0 = half * SH + sb * 512
                                for mt in range(DC // P):
                                    ps = p1ps.tile([P, 512], F32, tag="pqk")
                                    for kt in range(KT):
                                        nc.tensor.matmul(
                                            ps[:],
                                            w_sb[name][:, kt, mt * P:(mt + 1) * P],
                                            xT[:, kt, sb * 512:(sb + 1) * 512],
                                            start=(kt == 0),
                                            stop=(kt == KT - 1),
                                        )
                                    nc.vector.tensor_scalar_add(
                                        dst[:, mt, s0:s0 + 512],
                                        ps[:],
                                        bias[:, mt:mt + 1],
                                    )
                                    nc.vector.tensor_scalar_add(
                                        dstb[:, mt, s0:s0 + 512],
                                        ps[:],
                                        bias[:, mt:mt + 1],
                                    )
                        else:
                            for sc in range(SH // P):
                                ps = p1psv.tile([P, DC], F32, tag="pv")
                                for kt in range(KT):
                                    nc.tensor.matmul(
                                        ps[:],
                                        xT[:, kt, sc * P:(sc + 1) * P],
                                        w_sb["v"][:, kt, :],
                                        start=(kt == 0),
                                        stop=(kt == KT - 1),
                                    )
                                sk = half * (SH // P) + sc
                                for h in range(HPC):
                                    nc.vector.tensor_copy(
                                        v_aug[:, sk, h, 0:DK],
                                        ps[:, h * DK:(h + 1) * DK],
                                    )

            # ---------------- Phase 2: attention per head ----------------
            with (
                tc.tile_pool(name="p2e", bufs=2) as p2e,
                tc.tile_pool(name="p2et", bufs=2) as p2et,
                tc.tile_pool(name="p2s", bufs=2) as p2s,
                tc.tile_pool(name="psA", bufs=2, space="PSUM") as psA_pool,
                tc.tile_pool(name="psB", bufs=1, space="PSUM") as psB_pool,
                tc.tile_pool(name="psC", bufs=1, space="PSUM") as psC_pool,
                tc.tile_pool(name="psD", bufs=1, space="PSUM") as psD_pool,
            ):
                for h in range(HPC):
                    hp = (h % 2) * DK          # partition base within tile
                    hg = h // 2                # which 128-tile
                    qT_h = qT[hp:hp + DK, hg, :]
                    kT_h = kT[hp:hp + DK, hg, :]

                    # ---- pass A: scores [s_q, s_k] -> P output (f32r)
                    for ch in range(NCH):
                        E = p2e.tile([P, S], F32, tag="E")
                        dens = []
                        for half in range(2):
                            psA = psA_pool.tile([P, S // 2], F32, tag="A")
                            for nt in range(2):
                                o = half * 1024 + nt * 512
                                nc.tensor.matmul(
                                    psA[:, nt * 512:(nt + 1) * 512],
                                    qT_h[:, ch * P:(ch + 1) * P],
                                    kT_h[:, o:o + 512],
                                    start=True,
                                    stop=True,
                                )
                            # warm-keepers: dead matmuls into a scratch bank
                            # keep PE busy through the exp wait so HAM holds
                            # the 2.4 GHz clock (results never read)
                            psD = psD_pool.tile([P, 512], F32, tag="D")
                            for _w in range(2):
                                nc.tensor.matmul(
                                    psD[:],
                                    kTb[:, 0, 0:P],
                                    qTb[:, 0, 0:512],
                                    start=True,
                                    stop=True,
                                )
                            dh = p2s.tile([P, 1], F32, tag=f"den{half}")
                            nc.scalar.activation(
                                E[:, half * 1024:(half + 1) * 1024], psA[:],
                                mybir.ActivationFunctionType.Exp,
                                scale=float(1.0 / np.sqrt(DK)),
                                accum_out=dh[:],
                            )
                            dens.append(dh)
                        den = p2s.tile([P, 1], F32, tag="den")
                        nc.vector.tensor_add(den[:], dens[0][:], dens[1][:])
                        rec = p2s.tile([P, 1], F32, tag="rec")
                        nc.vector.reciprocal_approx_fast(rec[:], den[:])
                        nc.vector.tensor_scalar_mul(E[:], E[:], rec[:])
                        nc.sync.dma_start(
                            attw[h, ch * P:(ch + 1) * P, :], E[:]
                        )

                    qTb_h = qTb[hp:hp + DK, hg, :]
                    kTb_h = kTb[hp:hp + DK, hg, :]

                    # ---- pass B first: scores^T (bf16), PV + denom, so
                    # ctxT finishes before the last head's pass A and
                    # phase 3 overlaps the tail
                    for blk in range(NBLK):
                        ET = p2et.tile([P, NKT, BLK], BF16, tag="ET")
                        for kg in range(NKT // 2):
                            psB = psB_pool.tile([P, 2, BLK], F32, tag="B")
                            for i2 in range(2):
                                kt = kg * 2 + i2
                                nc.tensor.matmul(
                                    psB[:, i2, :],
                                    kTb_h[:, kt * P:(kt + 1) * P],
                                    qTb_h[:, blk * BLK:(blk + 1) * BLK],
                                    start=True,
                                    stop=True,
                                )
                            nc.scalar.activation(
                                ET[:, kg * 2:kg * 2 + 2, :], psB[:],
                                mybir.ActivationFunctionType.Exp,
                                scale=float(1.0 / np.sqrt(DK)),
                            )
                        psC = psC_pool.tile([VW, BLK], F32, tag="C")
                        for kt in range(NKT):
                            nc.tensor.matmul(
                                psC[:],
                                v_aug[:, kt, h, :],
                                ET[:, kt, :],
                                start=(kt == 0),
                                stop=(kt == NKT - 1),
                            )
                        df = p2s.tile([1, BLK], FP, tag="df")
                        nc.vector.tensor_copy(df[:], psC[DK:VW, :])
                        den_bc = p2s.tile([DK, BLK], FP, tag="denbc")
                        nc.gpsimd.partition_broadcast(den_bc[:], df[:])
                        rbc = p2s.tile([DK, BLK], FP, tag="rbc")
                        nc.vector.reciprocal_approx_fast(rbc[:], den_bc[:])
                        dstc = ctxT[hp:hp + DK, hg, blk * BLK:(blk + 1) * BLK]
                        ctmp = p2s.tile([DK, BLK], F32, tag="ctmp")
                        nc.vector.tensor_mul(ctmp[:], psC[0:DK, :], rbc[:])
                        nc.vector.tensor_scalar_add(
                            dstc, ctmp[:], bv_sb[hp:hp + DK, hg:hg + 1]
                        )

            # ---------------- Phase 3: output projection -----------------
            with (
                tc.tile_pool(name="psO", bufs=4, space="PSUM") as psO_pool,
                tc.tile_pool(name="p3sb", bufs=4) as p3sb,
            ):
                for ch in range(NCH):
                    psO = psO_pool.tile([P, D], F32, tag="O")
                    for half in range(2):
                        for g in range(DC // P):
                            nc.tensor.matmul(
                                psO[:, half * 512:(half + 1) * 512],
                                ctxT[:, g, ch * P:(ch + 1) * P],
                                wot_sb[:, g, half * 512:(half + 1) * 512],
                                start=(g == 0),
                                stop=(g == DC // P - 1),
                            )
                    o_sb = p3sb.tile([P, D], F32, tag="osb")
                    nc.vector.tensor_copy(o_sb[:], psO[:])
                    nc.sync.dma_start(
                        outp[ch * P:(ch + 1) * P, :], o_sb[:]
                    )

    nc.compile()
    return nc


_NC_CACHE = None


def _get_nc():
    global _NC_CACHE
    if _NC_CACHE is None:
        _NC_CACHE = build_nc()
    return _NC_CACHE


def _pmajor(a, kt):
    """[K, M] -> [128, kt, M] with K = t*128 + p, partition-major contiguous."""
    k, m = a.shape
    assert k == kt * P
    return np.ascontiguousarray(a.reshape(kt, P, m).transpose(1, 0, 2))


def make_in_maps(query, key_, value, W_q, b_q, W_k, b_k, W_v, b_v, W_o, b_o):
    query = np.asarray(query, dtype=np.float32)
    key_ = np.asarray(key_, dtype=np.float32)
    value = np.asarray(value, dtype=np.float32)
    W_q = np.asarray(W_q, dtype=np.float32)
    W_k = np.asarray(W_k, dtype=np.float32)
    W_v = np.asarray(W_v, dtype=np.float32)
    W_o = np.asarray(W_o, dtype=np.float32)
    # per-batch transposed activations [128, 2, 8, S//2]
    def _xarr(x):
        # [S, D] -> [p, half, kt, s'] with D = kt*128+p, s = half*S/2 + s'
        return np.ascontiguousarray(
            x.T.reshape(KT, P, 2, S // 2).transpose(1, 2, 0, 3)
        )

    xt = {}
    for b in range(B):
        xt[("q", b)] = _xarr(query[b])
        xt[("k", b)] = _xarr(key_[b])
        xt[("v", b)] = _xarr(value[b])
    in_maps = []
    for c in range(8):
        b = c // 4
        j = c % 4
        rows = slice(j * DC, (j + 1) * DC)
        in_maps.append({
            "xqt": xt[("q", b)],
            "xkt": xt[("k", b)],
            "xvt": xt[("v", b)],
            "wqt": _pmajor(np.ascontiguousarray(W_q[rows].T), KT),
            "wkt": _pmajor(np.ascontiguousarray(W_k[rows].T), KT),
            "wvt": _pmajor(np.ascontiguousarray(W_v[rows].T), KT),
            "wot": _pmajor(np.ascontiguousarray(W_o[:, rows].T), DC // P),
            "bq": np.asarray(b_q, np.float32)[rows].reshape(2, P).T.copy(),
            "bk": np.asarray(b_k, np.float32)[rows].reshape(2, P).T.copy(),
            "bv": np.asarray(b_v, np.float32)[rows].reshape(2, P).T.copy(),
        })
    return in_maps


def run(inputs, trace=False):
    _install_ntff_hook()
    nc = _get_nc()
    in_maps = make_in_maps(**inputs)
    res = run_bass_kernel_spmd(
        nc, in_maps, core_ids=list(range(8)), trace=trace
    )
    b_o = np.asarray(inputs["b_o"], np.float32)
    output = np.zeros((B, S, D), np.float32)
    attention_weights = np.empty((B, H, S, S), np.float32)
    for c in range(8):
        b = c // 4
        j = c % 4
        output[b] += res.results[c]["outp"]
        attention_weights[b, j * HPC:(j + 1) * HPC] = res.results[c]["attw"]
    output += b_o
    return (output, attention_weights), res


def kernel(**inputs):
    (output, attention_weights), _ = run(inputs, trace=False)
    return output, attention_weights


# revision 26
# speedup vs baseline: 1.0509x; 1.0509x over previous
"""Multi-head attention TRN2 kernel (B=2, S=2048, D=1024, H=16, DK=64).

Sharding: 8 cores; core c handles batch c//4 and heads 4*(c%4) .. 4*(c%4)+3.
Host pre-transposes activations/weights into partition-major layouts (pure
layout marshalling, no FLOPs); each core computes q/k/v projections for its
4 heads, full attention for those heads (including the [4, S, S]
attention-weight output), and a partial output projection. Host sums the 4
partial outputs per batch and adds b_o.
"""

import sys
import types

sys.path.insert(0, "/opt/trn_rl_repo")

import numpy as np

import concourse.bass as bass
import concourse.tile as tile
from concourse import bacc, mybir
from concourse.bass_utils import run_bass_kernel_spmd

B, S, D, H = 2, 2048, 1024, 16
DK = D // H          # 64
HPC = H // 4         # heads per core = 4
DC = HPC * DK        # 256 context dims per core
P = 128
KT = D // P          # 8 contraction tiles for projections

F32 = mybir.dt.float32
F32R = mybir.dt.float32r
BF16 = mybir.dt.bfloat16
FP = mybir.dt.float32

MMD = F32R  # dtype for f32-storage tiles consumed by matmul


def _install_ntff_hook():
    """run_bass_kernel_spmd(trace=True) needs antenv.axon_hooks, absent in
    this image. Recreate it and register the ctypes-based NTFF hook."""
    import antenv

    if "antenv.axon_hooks" in sys.modules:
        return
    mod = types.ModuleType("antenv.axon_hooks")
    mod._hook = None

    def set_axon_ntff_profile_hook(h):
        mod._hook = h

    def get_axon_ntff_profile_hook():
        return mod._hook

    mod.set_axon_ntff_profile_hook = set_axon_ntff_profile_hook
    mod.get_axon_ntff_profile_hook = get_axon_ntff_profile_hook
    sys.modules["antenv.axon_hooks"] = mod
    antenv.axon_hooks = mod
    try:
        from trn_agent_boot.trn_boot import _ntff_profile_via_ctypes

        hook = _ntff_profile_via_ctypes("/opt/axon/libaxon_pjrt.so")
        if hook is not None:
            set_axon_ntff_profile_hook(hook)
    except Exception:
        pass


def build_nc():
    nc = bacc.Bacc("TRN2", target_bir_lowering=False, debug=False)

    # host-marshalled, partition-major: [p, half, kt, s'] with D = kt*128 + p
    # and s = half*1024 + s'
    xqt = nc.dram_tensor("xqt", (P, 2, KT, S // 2), MMD, kind="ExternalInput")
    xkt = nc.dram_tensor("xkt", (P, 2, KT, S // 2), MMD, kind="ExternalInput")
    xvt = nc.dram_tensor("xvt", (P, 2, KT, S // 2), MMD, kind="ExternalInput")
    # weights [p, kt, dout] with D = kt*128 + p
    wqt = nc.dram_tensor("wqt", (P, KT, DC), MMD, kind="ExternalInput")
    wkt = nc.dram_tensor("wkt", (P, KT, DC), MMD, kind="ExternalInput")
    wvt = nc.dram_tensor("wvt", (P, KT, DC), MMD, kind="ExternalInput")
    # W_o slice [p, g, n] with d_core = g*128 + p
    wot = nc.dram_tensor("wot", (P, DC // P, D), MMD, kind="ExternalInput")
    bq = nc.dram_tensor("bq", (P, DC // P), FP, kind="ExternalInput")
    bk = nc.dram_tensor("bk", (P, DC // P), FP, kind="ExternalInput")
    bv = nc.dram_tensor("bv", (P, DC // P), FP, kind="ExternalInput")

    attw = nc.dram_tensor("attw", (HPC, S, S), FP, kind="ExternalOutput")
    outp = nc.dram_tensor("outp", (S, D), FP, kind="ExternalOutput")

    NCH = S // P       # 16 s_q chunks of 128 (pass A)
    NBLK = 4           # s_q blocks of 512 (pass B)
    BLK = S // NBLK    # 512
    NKT = S // P       # 16 s_k tiles
    VW = DK + 1        # 65: V columns + ones column

    with tile.TileContext(nc) as tc:
        with (
            tc.tile_pool(name="persist", bufs=1) as persist,
        ):
            # weights
            w_sb = {}
            for name, w in (("q", wqt), ("k", wkt), ("v", wvt)):
                t = persist.tile([P, KT, DC], MMD, tag=f"w{name}")
                nc.sync.dma_start(t[:], w[:])
                w_sb[name] = t
            wot_sb = persist.tile([P, DC // P, D], MMD, tag="wot")
            nc.sync.dma_start(wot_sb[:], wot[:])

            bq_sb = persist.tile([P, DC // P], FP, tag="bq")
            nc.sync.dma_start(bq_sb[:], bq[:])
            bk_sb = persist.tile([P, DC // P], FP, tag="bk")
            nc.sync.dma_start(bk_sb[:], bk[:])
            bv_sb = persist.tile([P, DC // P], FP, tag="bv")
            nc.sync.dma_start(bv_sb[:], bv[:])

            ones_sb = persist.tile([1, DK], FP, tag="ones")
            nc.vector.memset(ones_sb[:], 1.0)

            # persistent activations
            qT = persist.tile([P, DC // P, S], MMD, tag="qT")   # [d, s]
            kT = persist.tile([P, DC // P, S], MMD, tag="kT")
            # bf16 copies for pass-B score matmuls (f32r moving operands
            # stream at half rate; pass B feeds bf16 PV anyway)
            qTb = persist.tile([P, DC // P, S], BF16, tag="qTb")
            kTb = persist.tile([P, DC // P, S], BF16, tag="kTb")
            # [s_k(128), kt, h, VW] bf16: V columns + ones column per head
            v_aug = persist.tile([P, NKT, HPC, VW], BF16, tag="vaug")
            nc.vector.memset(v_aug[:, :, :, DK], 1.0)
            ctxT = persist.tile([P, DC // P, S], MMD, tag="ctxT")

            # ---------------- Phase 1: projections ----------------------
            with (
                tc.tile_pool(name="p1x", bufs=2) as p1x,
                tc.tile_pool(name="p1ps", bufs=2, space="PSUM") as p1ps,
                tc.tile_pool(name="p1psv", bufs=2, space="PSUM") as p1psv,
            ):
                SH = S // 2
                for name, x in (("q", xqt), ("k", xkt), ("v", xvt)):
                    for half in range(2):
                        xT = p1x.tile([P, KT, SH], MMD, tag="xT")
                        nc.sync.dma_start(xT[:], x[:, half, :, :])
                        if name in ("q", "k"):
                            dst = qT if name == "q" else kT
                            dstb = qTb if name == "q" else kTb
                            bias = bq_sb if name == "q" else bk_sb
                            for sb in range(SH // 512):
                                s0 = half * SH + sb * 512
                                for mt in range(DC // P):
                                    ps = p1ps.tile([P, 512], F32, tag="pqk")
                                    for kt in range(KT):
                                        nc.tensor.matmul(
                                            ps[:],
                                            w_sb[name][:, kt, mt * P:(mt + 1) * P],
                                            xT[:, kt, sb * 512:(sb + 1) * 512],
                                            start=(kt == 0),
                                            stop=(kt == KT - 1),
                                        )
                                    nc.vector.tensor_scalar_add(
                                        dst[:, mt, s0:s0 + 512],
                                        ps[:],
                                        bias[:, mt:mt + 1],
                                    )
                                    nc.vector.tensor_scalar_add(
                                        dstb[:, mt, s0:s0 + 512],
                                        ps[:],
                                        bias[:, mt:mt + 1],
                                    )
                        else:
                            for sc in range(SH // P):
                                ps = p1psv.tile([P, DC], F32, tag="pv")
                                for kt in range(KT):
                                    nc.tensor.matmul(
                                        ps[:],
                                        xT[:, kt, sc * P:(sc + 1) * P],
                                        w_sb["v"][:, kt, :],
                                        start=(kt == 0),
                                        stop=(kt == KT - 1),
                                    )
                                sk = half * (SH // P) + sc
                                for h in range(HPC):
                                    nc.vector.tensor_copy(
                                        v_aug[:, sk, h, 0:DK],
                                        ps[:, h * DK:(h + 1) * DK],
                                    )

            # ---------------- Phase 2: attention per head ----------------
            with (
                tc.tile_pool(name="p2e", bufs=2) as p2e,
                tc.tile_pool(name="p2et", bufs=2) as p2et,
                tc.tile_pool(name="p2s", bufs=2) as p2s,
                tc.tile_pool(name="psA", bufs=2, space="PSUM") as psA_pool,
                tc.tile_pool(name="psB", bufs=1, space="PSUM") as psB_pool,
                tc.tile_pool(name="psC", bufs=2, space="PSUM") as psC_pool,
            ):
                for h in range(HPC):
                    hp = (h % 2) * DK          # partition base within tile
                    hg = h // 2                # which 128-tile
                    qT_h = qT[hp:hp + DK, hg, :]
                    kT_h = kT[hp:hp + DK, hg, :]

                    # ---- pass A: scores [s_q, s_k] -> P output (f32r)
                    for ch in range(NCH):
                        E = p2e.tile([P, S], F32, tag="E")
                        dens = []
                        for half in range(2):
                            psA = psA_pool.tile([P, S // 2], F32, tag="A")
                            for nt in range(2):
                                o = half * 1024 + nt * 512
                                nc.tensor.matmul(
                                    psA[:, nt * 512:(nt + 1) * 512],
                                    qT_h[:, ch * P:(ch + 1) * P],
                                    kT_h[:, o:o + 512],
                                    start=True,
                                    stop=True,
                                )
                            dh = p2s.tile([P, 1], F32, tag=f"den{half}")
                            nc.scalar.activation(
                                E[:, half * 1024:(half + 1) * 1024], psA[:],
                                mybir.ActivationFunctionType.Exp,
                                scale=float(1.0 / np.sqrt(DK)),
                                accum_out=dh[:],
                            )
                            dens.append(dh)
                        den = p2s.tile([P, 1], F32, tag="den")
                        nc.vector.tensor_add(den[:], dens[0][:], dens[1][:])
                        rec = p2s.tile([P, 1], F32, tag="rec")
                        nc.vector.reciprocal_approx_fast(rec[:], den[:])
                        nc.vector.tensor_scalar_mul(E[:], E[:], rec[:])
                        nc.sync.dma_start(
                            attw[h, ch * P:(ch + 1) * P, :], E[:]
                        )

                    qTb_h = qTb[hp:hp + DK, hg, :]
                    kTb_h = kTb[hp:hp + DK, hg, :]

                    # ---- pass B first: scores^T (bf16), PV + denom, so
                    # ctxT finishes before the last head's pass A and
                    # phase 3 overlaps the tail
                    for blk in range(NBLK):
                        ET = p2et.tile([P, NKT, BLK], BF16, tag="ET")
                        for kg in range(NKT // 2):
                            psB = psB_pool.tile([P, 2, BLK], F32, tag="B")
                            for i2 in range(2):
                                kt = kg * 2 + i2
                                nc.tensor.matmul(
                                    psB[:, i2, :],
                                    kTb_h[:, kt * P:(kt + 1) * P],
                                    qTb_h[:, blk * BLK:(blk + 1) * BLK],
                                    start=True,
                                    stop=True,
                                )
                            nc.scalar.activation(
                                ET[:, kg * 2:kg * 2 + 2, :], psB[:],
                                mybir.ActivationFunctionType.Exp,
                                scale=float(1.0 / np.sqrt(DK)),
                            )
                        psC = psC_pool.tile([VW, BLK], F32, tag="C")
                        for kt in range(NKT):
                            nc.tensor.matmul(
                                psC[:],
                                v_aug[:, kt, h, :],
                                ET[:, kt, :],
                                start=(kt == 0),
                                stop=(kt == NKT - 1),
                            )
                        df = p2s.tile([1, BLK], FP, tag="df")
                        nc.vector.tensor_copy(df[:], psC[DK:VW, :])
                        den_bc = p2s.tile([DK, BLK], FP, tag="denbc")
                        nc.gpsimd.partition_broadcast(den_bc[:], df[:])
                        rbc = p2s.tile([DK, BLK], FP, tag="rbc")
                        nc.vector.reciprocal_approx_fast(rbc[:], den_bc[:])
                        dstc = ctxT[hp:hp + DK, hg, blk * BLK:(blk + 1) * BLK]
                        ctmp = p2s.tile([DK, BLK], F32, tag="ctmp")
                        nc.vector.tensor_mul(ctmp[:], psC[0:DK, :], rbc[:])
                        nc.vector.tensor_scalar_add(
                            dstc, ctmp[:], bv_sb[hp:hp + DK, hg:hg + 1]
                        )

            # ---------------- Phase 3: output projection -----------------
            with (
                tc.tile_pool(name="psO", bufs=4, space="PSUM") as psO_pool,
                tc.tile_pool(name="p3sb", bufs=4) as p3sb,
            ):
                for ch in range(NCH):
                    psO = psO_pool.tile([P, D], F32, tag="O")
                    for half in range(2):
                        for g in range(DC // P):
                            nc.tensor.matmul(
                                psO[:, half * 512:(half + 1) * 512],
                                ctxT[:, g, ch * P:(ch + 1) * P],
                                wot_sb[:, g, half * 512:(half + 1) * 512],
                                start=(g == 0),
                                stop=(g == DC // P - 1),
                            )
                    o_sb = p3sb.tile([P, D], F32, tag="osb")
                    nc.vector.tensor_copy(o_sb[:], psO[:])
                    nc.sync.dma_start(
                        outp[ch * P:(ch + 1) * P, :], o_sb[:]
                    )

    nc.compile()
    return nc


_NC_CACHE = None


def _get_nc():
    global _NC_CACHE
    if _NC_CACHE is None:
        _NC_CACHE = build_nc()
    return _NC_CACHE


def _pmajor(a, kt):
    """[K, M] -> [128, kt, M] with K = t*128 + p, partition-major contiguous."""
    k, m = a.shape
    assert k == kt * P
    return np.ascontiguousarray(a.reshape(kt, P, m).transpose(1, 0, 2))


def make_in_maps(query, key_, value, W_q, b_q, W_k, b_k, W_v, b_v, W_o, b_o):
    query = np.asarray(query, dtype=np.float32)
    key_ = np.asarray(key_, dtype=np.float32)
    value = np.asarray(value, dtype=np.float32)
    W_q = np.asarray(W_q, dtype=np.float32)
    W_k = np.asarray(W_k, dtype=np.float32)
    W_v = np.asarray(W_v, dtype=np.float32)
    W_o = np.asarray(W_o, dtype=np.float32)
    # per-batch transposed activations [128, 2, 8, S//2]
    def _xarr(x):
        # [S, D] -> [p, half, kt, s'] with D = kt*128+p, s = half*S/2 + s'
        return np.ascontiguousarray(
            x.T.reshape(KT, P, 2, S // 2).transpose(1, 2, 0, 3)
        )

    xt = {}
    for b in range(B):
        xt[("q", b)] = _xarr(query[b])
        xt[("k", b)] = _xarr(key_[b])
        xt[("v", b)] = _xarr(value[b])
    in_maps = []
    for c in range(8):
        b = c // 4
        j = c % 4
        rows = slice(j * DC, (j + 1) * DC)
        in_maps.append({
            "xqt": xt[("q", b)],
            "xkt": xt[("k", b)],
            "xvt": xt[("v", b)],
            "wqt": _pmajor(np.ascontiguousarray(W_q[rows].T), KT),
            "wkt": _pmajor(np.ascontiguousarray(W_k[rows].T), KT),
            "wvt": _pmajor(np.ascontiguousarray(W_v[rows].T), KT),
            "wot": _pmajor(np.ascontiguousarray(W_o[:, rows].T), DC // P),
            "bq": np.asarray(b_q, np.float32)[rows].reshape(2, P).T.copy(),
            "bk": np.asarray(b_k, np.float32)[rows].reshape(2, P).T.copy(),
            "bv": np.asarray(b_v, np.float32)[rows].reshape(2, P).T.copy(),
        })
    return in_maps


def run(inputs, trace=False):
    _install_ntff_hook()
    nc = _get_nc()
    in_maps = make_in_maps(**inputs)
    res = run_bass_kernel_spmd(
        nc, in_maps, core_ids=list(range(8)), trace=trace
    )
    b_o = np.asarray(inputs["b_o"], np.float32)
    output = np.zeros((B, S, D), np.float32)
    attention_weights = np.empty((B, H, S, S), np.float32)
    for c in range(8):
        b = c // 4
        j = c % 4
        output[b] += res.results[c]["outp"]
        attention_weights[b, j * HPC:(j + 1) * HPC] = res.results[c]["attw"]
    output += b_o
    return (output, attention_weights), res


def kernel(**inputs):
    (output, attention_weights), _ = run(inputs, trace=False)
    return output, attention_weights


# revision 27
# speedup vs baseline: 1.0763x; 1.0242x over previous
"""Multi-head attention TRN2 kernel (B=2, S=2048, D=1024, H=16, DK=64).

Sharding: 8 cores; core c handles batch c//4 and heads 4*(c%4) .. 4*(c%4)+3.
Host pre-transposes activations/weights into partition-major layouts (pure
layout marshalling, no FLOPs); each core computes q/k/v projections for its
4 heads, full attention for those heads (including the [4, S, S]
attention-weight output), and a partial output projection. Host sums the 4
partial outputs per batch and adds b_o.
"""

import sys
import types

sys.path.insert(0, "/opt/trn_rl_repo")

import numpy as np

import concourse.bass as bass
import concourse.tile as tile
from concourse import bacc, mybir
from concourse.bass_utils import run_bass_kernel_spmd

B, S, D, H = 2, 2048, 1024, 16
DK = D // H          # 64
HPC = H // 4         # heads per core = 4
DC = HPC * DK        # 256 context dims per core
P = 128
KT = D // P          # 8 contraction tiles for projections

F32 = mybir.dt.float32
F32R = mybir.dt.float32r
BF16 = mybir.dt.bfloat16
FP = mybir.dt.float32

MMD = F32R  # dtype for f32-storage tiles consumed by matmul


def _install_ntff_hook():
    """run_bass_kernel_spmd(trace=True) needs antenv.axon_hooks, absent in
    this image. Recreate it and register the ctypes-based NTFF hook."""
    import antenv

    if "antenv.axon_hooks" in sys.modules:
        return
    mod = types.ModuleType("antenv.axon_hooks")
    mod._hook = None

    def set_axon_ntff_profile_hook(h):
        mod._hook = h

    def get_axon_ntff_profile_hook():
        return mod._hook

    mod.set_axon_ntff_profile_hook = set_axon_ntff_profile_hook
    mod.get_axon_ntff_profile_hook = get_axon_ntff_profile_hook
    sys.modules["antenv.axon_hooks"] = mod
    antenv.axon_hooks = mod
    try:
        from trn_agent_boot.trn_boot import _ntff_profile_via_ctypes

        hook = _ntff_profile_via_ctypes("/opt/axon/libaxon_pjrt.so")
        if hook is not None:
            set_axon_ntff_profile_hook(hook)
    except Exception:
        pass


def build_nc():
    nc = bacc.Bacc("TRN2", target_bir_lowering=False, debug=False)

    # host-marshalled, partition-major: [p, half, kt, s'] with D = kt*128 + p
    # and s = half*1024 + s'
    xqt = nc.dram_tensor("xqt", (P, 2, KT, S // 2), MMD, kind="ExternalInput")
    xkt = nc.dram_tensor("xkt", (P, 2, KT, S // 2), MMD, kind="ExternalInput")
    xvt = nc.dram_tensor("xvt", (P, 2, KT, S // 2), MMD, kind="ExternalInput")
    # weights [p, kt, dout] with D = kt*128 + p
    wqt = nc.dram_tensor("wqt", (P, KT, DC), MMD, kind="ExternalInput")
    wkt = nc.dram_tensor("wkt", (P, KT, DC), MMD, kind="ExternalInput")
    wvt = nc.dram_tensor("wvt", (P, KT, DC), MMD, kind="ExternalInput")
    # W_o slice [p, g, n] with d_core = g*128 + p
    wot = nc.dram_tensor("wot", (P, DC // P, D), MMD, kind="ExternalInput")
    bq = nc.dram_tensor("bq", (P, DC // P), FP, kind="ExternalInput")
    bk = nc.dram_tensor("bk", (P, DC // P), FP, kind="ExternalInput")
    bv = nc.dram_tensor("bv", (P, DC // P), FP, kind="ExternalInput")

    attw = nc.dram_tensor("attw", (HPC, S, S), FP, kind="ExternalOutput")
    outp = nc.dram_tensor("outp", (S, D), FP, kind="ExternalOutput")

    NCH = S // P       # 16 s_q chunks of 128 (pass A)
    NBLK = 4           # s_q blocks of 512 (pass B)
    BLK = S // NBLK    # 512
    NKT = S // P       # 16 s_k tiles
    VW = DK + 1        # 65: V columns + ones column

    with tile.TileContext(nc) as tc:
        with (
            tc.tile_pool(name="persist", bufs=1) as persist,
        ):
            # weights
            w_sb = {}
            for name, w in (("q", wqt), ("k", wkt), ("v", wvt)):
                t = persist.tile([P, KT, DC], MMD, tag=f"w{name}")
                nc.sync.dma_start(t[:], w[:])
                w_sb[name] = t
            wot_sb = persist.tile([P, DC // P, D], MMD, tag="wot")
            nc.sync.dma_start(wot_sb[:], wot[:])

            bq_sb = persist.tile([P, DC // P], FP, tag="bq")
            nc.sync.dma_start(bq_sb[:], bq[:])
            bk_sb = persist.tile([P, DC // P], FP, tag="bk")
            nc.sync.dma_start(bk_sb[:], bk[:])
            bv_sb = persist.tile([P, DC // P], FP, tag="bv")
            nc.sync.dma_start(bv_sb[:], bv[:])

            ones_sb = persist.tile([1, DK], FP, tag="ones")
            nc.vector.memset(ones_sb[:], 1.0)

            # persistent activations
            qT = persist.tile([P, DC // P, S], MMD, tag="qT")   # [d, s]
            kT = persist.tile([P, DC // P, S], MMD, tag="kT")
            # bf16 copies for pass-B score matmuls (f32r moving operands
            # stream at half rate; pass B feeds bf16 PV anyway)
            qTb = persist.tile([P, DC // P, S], BF16, tag="qTb")
            kTb = persist.tile([P, DC // P, S], BF16, tag="kTb")
            # [s_k(128), kt, h, VW] bf16: V columns + ones column per head
            v_aug = persist.tile([P, NKT, HPC, VW], BF16, tag="vaug")
            nc.vector.memset(v_aug[:, :, :, DK], 1.0)
            ctxT = persist.tile([P, DC // P, S], MMD, tag="ctxT")

            # ---------------- Phase 1: projections ----------------------
            with (
                tc.tile_pool(name="p1x", bufs=2) as p1x,
                tc.tile_pool(name="p1ps", bufs=2, space="PSUM") as p1ps,
                tc.tile_pool(name="p1psv", bufs=2, space="PSUM") as p1psv,
            ):
                SH = S // 2
                for name, x in (("q", xqt), ("k", xkt), ("v", xvt)):
                    for half in range(2):
                        xT = p1x.tile([P, KT, SH], MMD, tag="xT")
                        nc.sync.dma_start(xT[:], x[:, half, :, :])
                        if name in ("q", "k"):
                            dst = qT if name == "q" else kT
                            dstb = qTb if name == "q" else kTb
                            bias = bq_sb if name == "q" else bk_sb
                            for sb in range(SH // 512):
                                s0 = half * SH + sb * 512
                                for mt in range(DC // P):
                                    ps = p1ps.tile([P, 512], F32, tag="pqk")
                                    for kt in range(KT):
                                        nc.tensor.matmul(
                                            ps[:],
                                            w_sb[name][:, kt, mt * P:(mt + 1) * P],
                                            xT[:, kt, sb * 512:(sb + 1) * 512],
                                            start=(kt == 0),
                                            stop=(kt == KT - 1),
                                        )
                                    nc.vector.tensor_scalar_add(
                                        dst[:, mt, s0:s0 + 512],
                                        ps[:],
                                        bias[:, mt:mt + 1],
                                    )
                                    nc.vector.tensor_scalar_add(
                                        dstb[:, mt, s0:s0 + 512],
                                        ps[:],
                                        bias[:, mt:mt + 1],
                                    )
                        else:
                            for sc in range(SH // P):
                                ps = p1psv.tile([P, DC], F32, tag="pv")
                                for kt in range(KT):
                                    nc.tensor.matmul(
                                        ps[:],
                                        xT[:, kt, sc * P:(sc + 1) * P],
                                        w_sb["v"][:, kt, :],
                                        start=(kt == 0),
                                        stop=(kt == KT - 1),
                                    )
                                sk = half * (SH // P) + sc
                                for h in range(HPC):
                                    nc.vector.tensor_copy(
                                        v_aug[:, sk, h, 0:DK],
                                        ps[:, h * DK:(h + 1) * DK],
                                    )

            # ---------------- Phase 2: attention per head ----------------
            with (
                tc.tile_pool(name="p2e", bufs=3) as p2e,
                tc.tile_pool(name="p2et", bufs=2) as p2et,
                tc.tile_pool(name="p2s", bufs=3) as p2s,
                tc.tile_pool(name="psA", bufs=2, space="PSUM") as psA_pool,
                tc.tile_pool(name="psB", bufs=1, space="PSUM") as psB_pool,
                tc.tile_pool(name="psC", bufs=2, space="PSUM") as psC_pool,
            ):
                for h in range(HPC):
                    hp = (h % 2) * DK          # partition base within tile
                    hg = h // 2                # which 128-tile
                    qT_h = qT[hp:hp + DK, hg, :]
                    kT_h = kT[hp:hp + DK, hg, :]

                    # ---- pass A: scores [s_q, s_k] -> P output (f32r)
                    for ch in range(NCH):
                        E = p2e.tile([P, S], F32, tag="E")
                        dens = []
                        for half in range(2):
                            psA = psA_pool.tile([P, S // 2], F32, tag="A")
                            for nt in range(2):
                                o = half * 1024 + nt * 512
                                nc.tensor.matmul(
                                    psA[:, nt * 512:(nt + 1) * 512],
                                    qT_h[:, ch * P:(ch + 1) * P],
                                    kT_h[:, o:o + 512],
                                    start=True,
                                    stop=True,
                                )
                            dh = p2s.tile([P, 1], F32, tag=f"den{half}")
                            nc.scalar.activation(
                                E[:, half * 1024:(half + 1) * 1024], psA[:],
                                mybir.ActivationFunctionType.Exp,
                                scale=float(1.0 / np.sqrt(DK)),
                                accum_out=dh[:],
                            )
                            dens.append(dh)
                        den = p2s.tile([P, 1], F32, tag="den")
                        nc.vector.tensor_add(den[:], dens[0][:], dens[1][:])
                        rec = p2s.tile([P, 1], F32, tag="rec")
                        nc.vector.reciprocal_approx_fast(rec[:], den[:])
                        nc.vector.tensor_scalar_mul(E[:], E[:], rec[:])
                        nc.sync.dma_start(
                            attw[h, ch * P:(ch + 1) * P, :], E[:]
                        )

                    qTb_h = qTb[hp:hp + DK, hg, :]
                    kTb_h = kTb[hp:hp + DK, hg, :]

                    # ---- pass B first: scores^T (bf16), PV + denom, so
                    # ctxT finishes before the last head's pass A and
                    # phase 3 overlaps the tail
                    for blk in range(NBLK):
                        ET = p2et.tile([P, NKT, BLK], BF16, tag="ET")
                        for kg in range(NKT // 2):
                            psB = psB_pool.tile([P, 2, BLK], F32, tag="B")
                            for i2 in range(2):
                                kt = kg * 2 + i2
                                nc.tensor.matmul(
                                    psB[:, i2, :],
                                    kTb_h[:, kt * P:(kt + 1) * P],
                                    qTb_h[:, blk * BLK:(blk + 1) * BLK],
                                    start=True,
                                    stop=True,
                                )
                            nc.scalar.activation(
                                ET[:, kg * 2:kg * 2 + 2, :], psB[:],
                                mybir.ActivationFunctionType.Exp,
                                scale=float(1.0 / np.sqrt(DK)),
                            )
                        psC = psC_pool.tile([VW, BLK], F32, tag="C")
                        for kt in range(NKT):
                            nc.tensor.matmul(
                                psC[:],
                                v_aug[:, kt, h, :],
                                ET[:, kt, :],
                                start=(kt == 0),
                                stop=(kt == NKT - 1),
                            )
                        df = p2s.tile([1, BLK], FP, tag="df")
                        nc.vector.tensor_copy(df[:], psC[DK:VW, :])
                        den_bc = p2s.tile([DK, BLK], FP, tag="denbc")
                        nc.gpsimd.partition_broadcast(den_bc[:], df[:])
                        rbc = p2s.tile([DK, BLK], FP, tag="rbc")
                        nc.vector.reciprocal_approx_fast(rbc[:], den_bc[:])
                        dstc = ctxT[hp:hp + DK, hg, blk * BLK:(blk + 1) * BLK]
                        ctmp = p2s.tile([DK, BLK], F32, tag="ctmp")
                        nc.vector.tensor_mul(ctmp[:], psC[0:DK, :], rbc[:])
                        nc.vector.tensor_scalar_add(
                            dstc, ctmp[:], bv_sb[hp:hp + DK, hg:hg + 1]
                        )

            # ---------------- Phase 3: output projection -----------------
            with (
                tc.tile_pool(name="psO", bufs=4, space="PSUM") as psO_pool,
                tc.tile_pool(name="p3sb", bufs=4) as p3sb,
            ):
                for ch in range(NCH):
                    psO = psO_pool.tile([P, D], F32, tag="O")
                    for half in range(2):
                        for g in range(DC // P):
                            nc.tensor.matmul(
                                psO[:, half * 512:(half + 1) * 512],
                                ctxT[:, g, ch * P:(ch + 1) * P],
                                wot_sb[:, g, half * 512:(half + 1) * 512],
                                start=(g == 0),
                                stop=(g == DC // P - 1),
                            )
                    o_sb = p3sb.tile([P, D], F32, tag="osb")
                    nc.vector.tensor_copy(o_sb[:], psO[:])
                    nc.sync.dma_start(
                        outp[ch * P:(ch + 1) * P, :], o_sb[:]
                    )

    nc.compile()
    return nc


_NC_CACHE = None


def _get_nc():
    global _NC_CACHE
    if _NC_CACHE is None:
        _NC_CACHE = build_nc()
    return _NC_CACHE


def _pmajor(a, kt):
    """[K, M] -> [128, kt, M] with K = t*128 + p, partition-major contiguous."""
    k, m = a.shape
    assert k == kt * P
    return np.ascontiguousarray(a.reshape(kt, P, m).transpose(1, 0, 2))


def make_in_maps(query, key_, value, W_q, b_q, W_k, b_k, W_v, b_v, W_o, b_o):
    query = np.asarray(query, dtype=np.float32)
    key_ = np.asarray(key_, dtype=np.float32)
    value = np.asarray(value, dtype=np.float32)
    W_q = np.asarray(W_q, dtype=np.float32)
    W_k = np.asarray(W_k, dtype=np.float32)
    W_v = np.asarray(W_v, dtype=np.float32)
    W_o = np.asarray(W_o, dtype=np.float32)
    # per-batch transposed activations [128, 2, 8, S//2]
    def _xarr(x):
        # [S, D] -> [p, half, kt, s'] with D = kt*128+p, s = half*S/2 + s'
        return np.ascontiguousarray(
            x.T.reshape(KT, P, 2, S // 2).transpose(1, 2, 0, 3)
        )

    xt = {}
    for b in range(B):
        xt[("q", b)] = _xarr(query[b])
        xt[("k", b)] = _xarr(key_[b])
        xt[("v", b)] = _xarr(value[b])
    in_maps = []
    for c in range(8):
        b = c // 4
        j = c % 4
        rows = slice(j * DC, (j + 1) * DC)
        in_maps.append({
            "xqt": xt[("q", b)],
            "xkt": xt[("k", b)],
            "xvt": xt[("v", b)],
            "wqt": _pmajor(np.ascontiguousarray(W_q[rows].T), KT),
            "wkt": _pmajor(np.ascontiguousarray(W_k[rows].T), KT),
            "wvt": _pmajor(np.ascontiguousarray(W_v[rows].T), KT),
            "wot": _pmajor(np.ascontiguousarray(W_o[:, rows].T), DC // P),
            "bq": np.asarray(b_q, np.float32)[rows].reshape(2, P).T.copy(),
            "bk": np.asarray(b_k, np.float32)[rows].reshape(2, P).T.copy(),
            "bv": np.asarray(b_v, np.float32)[rows].reshape(2, P).T.copy(),
        })
    return in_maps


def run(inputs, trace=False):
    _install_ntff_hook()
    nc = _get_nc()
    in_maps = make_in_maps(**inputs)
    res = run_bass_kernel_spmd(
        nc, in_maps, core_ids=list(range(8)), trace=trace
    )
    b_o = np.asarray(inputs["b_o"], np.float32)
    output = np.zeros((B, S, D), np.float32)
    attention_weights = np.empty((B, H, S, S), np.float32)
    for c in range(8):
        b = c // 4
        j = c % 4
        output[b] += res.results[c]["outp"]
        attention_weights[b, j * HPC:(j + 1) * HPC] = res.results[c]["attw"]
    output += b_o
    return (output, attention_weights), res


def kernel(**inputs):
    (output, attention_weights), _ = run(inputs, trace=False)
    return output, attention_weights
